# revision 20
# baseline (speedup 1.0000x reference)
"""Trainium2 Bass kernel for the low-rank MGD (Mahalanobis Gaussian) loss.

Strategy (data-parallel over batch across 8 NeuronCores):
  - Each core receives a [384, 4000] shard of x (384 = 16 samples x 24
    q-rows), computes per-row sums of x^2 (fused DVE multiply-reduce) and
    Y^T = Ln_s^T @ X^T ([30, 384]) via PE transpose + PSUM-accumulated
    matmuls over 32 n-chunks of 125.
  - The y_t != 0 mask is handled on the host: y_t is randn-filled, so it
    contains an exact f32 zero with probability ~0; kernel() verifies that
    and falls back to masking x on the host in the degenerate case. The
    device therefore only streams x (49MB instead of 98MB).
  - Host gathers the tiny per-core outputs and finishes: z = Lq_s^T @ Y_b,
    the 360x360 capacitance cholesky / logdet / triangular solve, and the
    final scalar loss. This is ~30 MFLOP of O(R^3) linear algebra on 47KB
    of data - negligible next to what the device streams.
"""

import os
import sys
import types
import contextlib
from contextlib import ExitStack

import numpy as np

if "/opt/trn_rl_repo" not in sys.path:
    sys.path.insert(0, "/opt/trn_rl_repo")

import concourse.bass as bass
import concourse.tile as tile
import concourse.mybir as mybir
from concourse.bass_utils import run_bass_kernel_spmd
from concourse.vector_clock import ScopedClock

F32 = mybir.dt.float32

# Problem constants (hardcoded per the harness contract).
B, Q, N = 128, 24, 4000
RANK_N, RANK_Q = 30, 12
SIGMA_INIT = 1.0
SIGMA_MIN = 0.001
NCORES = 8
BSH = B // NCORES          # samples per core = 16
ROWS = BSH * Q             # (b, q) rows per core = 384
RT = ROWS // 128           # 128-row tiles per core = 3
NH = 8                     # DMA/DVE column phases of 512 (last 416)
HCH = 512
NCH = 32                   # matmul n-chunks of 128 (last 32)
CH = 128
CPH = 4                    # n-chunks per phase

LAST_EXEC_TIME_NS = None


# ---------------------------------------------------------------------------
# Environment fixups
# ---------------------------------------------------------------------------

_MAX_WAITS = 1  # walrus codegen here rejects multiple sync-waits on one instruction


def _apply_tile_wait_split_patch():
    """walrus in this image rejects >2 sync-waits on one instruction
    ("Too many sync wait commands"). Split excess waits onto same-engine
    nops placed immediately before the over-subscribed instruction, and
    do the same for the Tile tail Drain."""
    if getattr(tile.TileContext, "_wait_split_applied", False):
        return

    orig_lower = tile.TileContext._lower_ordered_insts

    def _split_waits(self, ordered):
        for bb_name, insts in ordered.items():
            out = []
            for inst in insts:
                si = inst.sync_info
                if si is not None and len(si.on_wait) > _MAX_WAITS:
                    waits = list(si.on_wait)
                    rest, keep = waits[:-_MAX_WAITS], waits[-_MAX_WAITS:]
                    inst.sync_info = mybir.SyncInfo(
                        on_update=list(si.on_update), on_wait=keep
                    )
                    for i in range(0, len(rest), _MAX_WAITS):
                        out.append(
                            mybir.InstNoOp(
                                name=f"{inst.name}.wsplit{i}",
                                engine=inst.engine,
                                bass_nofuse=True,
                                sync_info=mybir.SyncInfo(
                                    on_update=[],
                                    on_wait=rest[i : i + _MAX_WAITS],
                                ),
                            )
                        )
                out.append(inst)
            ordered[bb_name] = out

    def _lower_ordered_insts(self, ordered):
        _split_waits(self, ordered)
        return orig_lower(self, ordered)

    def _drain_and_barrier(self, tick_clock, wait_clock):
        drain_inst = self.nc.sync.drain()
        wait_clock.add_sem_waits(
            drain_inst.ins, ScopedClock({None: tick_clock.global_clock})
        )
        waits = list(drain_inst.ins.sync_info.on_wait)
        if len(waits) > _MAX_WAITS:
            drain_inst.ins.sync_info.on_wait = waits[:_MAX_WAITS]
            rest = waits[_MAX_WAITS:]
            for i in range(0, len(rest), _MAX_WAITS):
                nop = self.nc.sync.nop(nofuse=True, hint="drain_wait_split")
                nop.ins.sync_info = mybir.SyncInfo(
                    on_update=[], on_wait=rest[i : i + _MAX_WAITS]
                )

        self.nc.all_engine_barrier()
        assert self.sems is not None
        popped = self.nc._tile_sem_poison_stack.pop()
        assert popped is self._sem_poison
        self.nc.clear_and_free_semaphores(list(self.sems.allocated().values()))
        self.nc.all_engine_barrier()

    tile.TileContext._lower_ordered_insts = _lower_ordered_insts
    tile.TileContext._drain_and_barrier = _drain_and_barrier
    tile.TileContext._wait_split_applied = True


def _install_ntff_hook():
    """Register the axon NTFF profile hook (the image's antenv package lacks
    axon_hooks, so trace=True would silently degrade otherwise)."""
    if "antenv.axon_hooks" in sys.modules:
        return
    mod = types.ModuleType("antenv.axon_hooks")
    state = {"hook": None}
    mod.set_axon_ntff_profile_hook = lambda h: state.__setitem__("hook", h)
    mod.get_axon_ntff_profile_hook = lambda: state["hook"]
    sys.modules["antenv.axon_hooks"] = mod
    try:
        import antenv

        antenv.axon_hooks = mod
    except Exception:
        pass
    try:
        from trn_agent_boot.trn_boot import _ntff_profile_via_ctypes

        hook = _ntff_profile_via_ctypes("/opt/axon/libaxon_pjrt.so")
        if hook is not None:
            mod.set_axon_ntff_profile_hook(hook)
    except Exception:
        pass


_apply_tile_wait_split_patch()
_install_ntff_hook()


# ---------------------------------------------------------------------------
# Device kernel
# ---------------------------------------------------------------------------

ZW = BSH * RANK_Q          # z^T columns per core = 192
BF16 = mybir.dt.bfloat16


def _phase_cols(h):
    return min(HCH, N - HCH * h)


def _chunk_cols(c):
    return min(CH, N - CH * c)


def _build_nc():
    """Per core: z^T = sum_n sum_q x[(s,q), n] Lq_s[q, i] Ln_s[n, j].

    Stage 1 (per n-chunk c of 128): psum_T[n', (s,i)] accumulates
    x_tile_r^T @ BD_r over the 3 row-tiles r, where BD_r is the
    block-diagonal Lq_s for the samples covered by rows [128r, 128r+128).
    Samples straddling a row-tile boundary are summed by the PSUM
    accumulation. x tiles are the stationary operand in natural layout
    (no transposes; every PE op is a real matmul), converted to bf16 so
    LDWEIGHTS runs with FWL and overlaps in-flight matmuls.

    Stage 2: psum_z[j, (s,i)] accumulates lns_c^T @ T_c over the 32
    chunks. Stage-2 matmuls are emitted DELAY chunks behind stage 1 so
    the PSUM->SBUF copy of T_c is off the PE critical path.
    """
    nc = bass.Bass()
    x = nc.declare_dram_parameter("x", [ROWS, N], F32, isOutput=False)
    lns = nc.declare_dram_parameter("lns", [128, NCH * RANK_N], BF16, isOutput=False)
    bd = nc.declare_dram_parameter("bd", [128, RT * ZW], BF16, isOutput=False)
    zt = nc.declare_dram_parameter("zt", [RANK_N, ZW], F32, isOutput=True)
    rs = nc.declare_dram_parameter("rs", [128, RT * NH], F32, isOutput=True)

    mult = mybir.AluOpType.mult
    DELAY = 2

    with tile.TileContext(nc) as tc, ExitStack() as ctx:
        const = ctx.enter_context(tc.tile_pool(name="const", bufs=1))
        bfp = [
            ctx.enter_context(tc.tile_pool(name=f"bf{r}", bufs=3)) for r in range(RT)
        ]
        sqp = ctx.enter_context(tc.tile_pool(name="sq", bufs=2))
        ttp = ctx.enter_context(tc.tile_pool(name="tt", bufs=DELAY + 2))
        outp = ctx.enter_context(tc.tile_pool(name="outs", bufs=1))
        pt = ctx.enter_context(tc.tile_pool(name="pt", bufs=DELAY + 2, space="PSUM"))
        pz = ctx.enter_context(tc.tile_pool(name="pz", bufs=1, space="PSUM"))

        bd_sb = const.tile([128, RT * ZW], BF16)
        nc.sync.dma_start(bd_sb[:], bd[:])
        # lns chunk c ([<=128, 30]) lives at columns [30c, 30c+30),
        # row-packed on the host.
        lns_sb = const.tile([128, NCH * RANK_N], BF16)
        nc.sync.dma_start(lns_sb[:], lns[:])
        rs_sb = outp.tile([128, RT * NH], F32)

        pzt = pz.tile([RANK_N, ZW], F32)
        pending = []  # (chunk, tt tile) awaiting the stage-2 matmul

        def stage2(c, tt):
            csz = _chunk_cols(c)
            nc.tensor.matmul(
                pzt[:],
                lns_sb[0:csz, RANK_N * c : RANK_N * (c + 1)],
                tt[0:csz, :],
                start=(c == 0),
                stop=(c == NCH - 1),
            )

        for h in range(NH):
            pcols = _phase_cols(h)
            xbf = [None] * RT
            for r in range(RT):
                # Casting DMA (SWDGE): f32 DRAM -> bf16 SBUF in one shot.
                xb = bfp[r].tile([128, HCH], BF16)
                nc.gpsimd.dma_start(
                    xb[0:128, 0:pcols],
                    x[128 * r : 128 * (r + 1), HCH * h : HCH * h + pcols],
                )
                xbf[r] = xb
                # x^2 (to scratch) + rowsum accumulator in one DVE op
                sq = sqp.tile([128, HCH], BF16)
                slot = r * NH + h
                nc.vector.scalar_tensor_tensor(
                    sq[0:128, 0:pcols],
                    xb[0:128, 0:pcols],
                    1.0,
                    xb[0:128, 0:pcols],
                    mult,
                    mult,
                    accum_out=rs_sb[:, slot : slot + 1],
                )
            for cc in range(CPH):
                c = h * CPH + cc
                csz = _chunk_cols(c)
                ptc = pt.tile([CH, ZW], F32)
                for r in range(RT):
                    nc.tensor.matmul(
                        ptc[0:csz, :],
                        xbf[r][:, CH * cc : CH * cc + csz],
                        bd_sb[:, ZW * r : ZW * (r + 1)],
                        start=(r == 0),
                        stop=(r == RT - 1),
                    )
                tt = ttp.tile([CH, ZW], BF16)
                # PSUM->SBUF copies on ScalarE (otherwise idle).
                nc.scalar.copy(tt[0:csz, :], ptc[0:csz, :])
                pending.append((c, tt))
                if len(pending) > DELAY:
                    stage2(*pending.pop(0))
        for c, tt in pending:
            stage2(c, tt)

        zto = outp.tile([RANK_N, ZW], F32, tag="zto")
        nc.scalar.copy(zto[:], pzt[:])
        nc.sync.dma_start(zt[:], zto[:])
        # Copy through DVE (program order after all accum writers) so the
        # DMA-out has a tracked producer for every element.
        rs_out = outp.tile([128, RT * NH], F32, tag="rs_out")
        nc.vector.tensor_copy(rs_out[:], rs_sb[:])
        nc.sync.dma_start(rs[:], rs_out[:])
    return nc


_NC = None


def _get_nc():
    global _NC
    if _NC is None:
        _NC = _build_nc()
    return _NC


# ---------------------------------------------------------------------------
# Host wrapper
# ---------------------------------------------------------------------------

def kernel(eps_t, y_t, L_n, L_q, sigma):
    global LAST_EXEC_TIME_NS
    eps_t = np.ascontiguousarray(eps_t, dtype=np.float32)
    y_t = np.ascontiguousarray(y_t, dtype=np.float32)
    L_n = np.asarray(L_n, dtype=np.float32)
    L_q = np.asarray(L_q, dtype=np.float32)
    sigma = np.asarray(sigma, dtype=np.float32)
    assert eps_t.shape == (B, Q, N) and y_t.shape == (B, Q, N)

    import ml_dtypes

    lns = np.ascontiguousarray(L_n / np.float32(np.sqrt(RANK_N)))
    lqs32 = (L_q / np.float32(np.sqrt(RANK_Q))).astype(np.float32)
    lqs = lqs32.astype(np.float64)

    # lns row-packed into chunks of 128: lnp[p, 30c + j] = lns[128c + p, j]
    lnp = np.zeros((128, NCH * RANK_N), dtype=np.float32)
    for c in range(NCH):
        csz = _chunk_cols(c)
        lnp[:csz, RANK_N * c : RANK_N * (c + 1)] = lns[CH * c : CH * c + csz]
    lnp = lnp.astype(ml_dtypes.bfloat16)

    # Block-diagonal Lq_s per 128-row tile: bd[p, r*ZW + s*12 + i] =
    # lqs[q, i] where 128r + p = 24s + q (sample-local rows).
    bdm = np.zeros((128, RT * ZW), dtype=np.float32)
    for r in range(RT):
        for p in range(128):
            g = 128 * r + p
            s, q = divmod(g, Q)
            bdm[p, r * ZW + s * RANK_Q : r * ZW + (s + 1) * RANK_Q] = lqs32[q]
    bdm = bdm.astype(ml_dtypes.bfloat16)

    # The reference masks x where y_t is exactly 0.0f. y_t is randn-filled,
    # so this never fires in practice; handle the degenerate case on the
    # host so the device only has to stream x.
    if np.any(y_t == 0.0):
        eps_t = eps_t * (y_t != 0.0).astype(np.float32)

    xf = eps_t.reshape(B * Q, N)
    in_maps = [
        {
            "x": np.ascontiguousarray(xf[i * ROWS : (i + 1) * ROWS]),
            "lns": lnp,
            "bd": bdm,
        }
        for i in range(NCORES)
    ]

    nc = _get_nc()
    trace = bool(os.environ.get("BASS_KERNEL_TRACE"))
    res = run_bass_kernel_spmd(nc, in_maps, list(range(NCORES)), trace=trace)
    if trace:
        LAST_EXEC_TIME_NS = res.exec_time_ns

    # Gather z [B, R] (device zt is [30, (s, i)] per core) and row sums.
    z = np.concatenate(
        [
            res.results[i]["zt"]
            .astype(np.float64)
            .reshape(RANK_N, BSH, RANK_Q)
            .transpose(1, 2, 0)
            .reshape(BSH, RANK_Q * RANK_N)
            for i in range(NCORES)
        ]
    )
    rows = np.concatenate(
        [
            res.results[i]["rs"].reshape(128, RT, NH).sum(axis=2).T.reshape(ROWS)
            for i in range(NCORES)
        ]
    )

    return _host_finish(z, rows, lqs, lns.astype(np.float64), sigma)


def _host_finish(z, rows, lqs, lns64, sigma):
    """Tiny O(R^3) finish in float64. z: [B, R]; rows: [B*Q] sums of
    masked x^2; lqs/lns64: scaled cov factors in float64."""
    D = Q * N
    R = RANK_Q * RANK_N

    s2 = rows.astype(np.float64).reshape(B, Q).sum(axis=1)

    # Capacitance grams: A = lqs^T lqs (rq x rq), Bm = lns^T lns (rn x rn).
    A = lqs.T @ lqs
    Bm = lns64.T @ lns64

    diag_bias = np.log(np.expm1(np.float64(SIGMA_INIT**2)))
    c = np.logaddexp(0.0, np.float64(sigma[0]) + diag_bias) + SIGMA_MIN**2

    cap = np.eye(R) + np.kron(A, Bm) / c
    L = np.linalg.cholesky(cap)
    logdet = 2.0 * np.sum(np.log(np.diagonal(L))) + D * np.log(c)

    try:
        from scipy.linalg import solve_triangular

        u = solve_triangular(L, z.T, lower=True)
    except Exception:
        u = np.linalg.solve(L, z.T)
    maha = s2 / c - (u * u).sum(axis=0) / (c * c)

    loss = np.mean(0.5 * (D * np.log(2.0 * np.pi) + logdet + maha))
    return np.float32(loss)


# revision 48
# speedup vs baseline: 1.1384x; 1.1384x over previous
"""Trainium2 Bass kernel for the low-rank MGD (Mahalanobis Gaussian) loss.

Strategy (data-parallel over batch across 8 NeuronCores):
  - Each core receives a [384, 4000] shard of x (384 = 16 samples x 24
    q-rows) and computes, fully on device, its samples' Mahalanobis
    ingredients: per-row sums of x^2 (fused DVE multiply-reduce) and
    z^T[j, (s,i)] = sum_{n,q} x[(s,q),n] Lq_s[q,i] Ln_s[n,j] via two
    PSUM-accumulated matmul stages (x as the bf16 stationary operand
    against a block-diagonal Lq_s, then Ln_s^T against the stage-1
    output). No transposes are needed anywhere.
  - The y_t != 0 mask is handled on the host: y_t is randn-filled, so it
    contains an exact f32 zero with probability ~0; kernel() verifies that
    and falls back to masking x on the host in the degenerate case. The
    device therefore only streams x (49MB instead of 98MB).
  - Host gathers the tiny per-core outputs (z [B, 360] and row sums) and
    finishes: the 360x360 capacitance cholesky / logdet / triangular
    solve, and the final scalar loss. This is ~30 MFLOP of O(R^3) linear
    algebra on 47KB of data - negligible next to what the device streams.
"""

import os
import sys
import types
from contextlib import ExitStack

import numpy as np

if "/opt/trn_rl_repo" not in sys.path:
    sys.path.insert(0, "/opt/trn_rl_repo")

import concourse.bass as bass
import concourse.tile as tile
import concourse.mybir as mybir
from concourse.bass_utils import run_bass_kernel_spmd
from concourse.vector_clock import ScopedClock

F32 = mybir.dt.float32

# Problem constants (hardcoded per the harness contract).
B, Q, N = 128, 24, 4000
RANK_N, RANK_Q = 30, 12
SIGMA_INIT = 1.0
SIGMA_MIN = 0.001
NCORES = 8
BSH = B // NCORES          # samples per core = 16
ROWS = BSH * Q             # (b, q) rows per core = 384
RT = ROWS // 128           # 128-row tiles per core = 3
NCH = 32                   # matmul n-chunks of 128 (last 32)
CH = 128
# chunks per phase: small first (fast pipeline fill), big in the middle
# (few triggers at steady state), small last (fast drain)
PH = [4, 8, 8, 8, 2, 1, 1]
NPH = len(PH)
PH_OFF = [sum(PH[:i]) for i in range(NPH)]      # first chunk of each phase

LAST_EXEC_TIME_NS = None


# ---------------------------------------------------------------------------
# Environment fixups
# ---------------------------------------------------------------------------

_MAX_WAITS = 1  # walrus codegen here rejects multiple sync-waits on one instruction


def _apply_tile_wait_split_patch():
    """walrus in this image rejects >2 sync-waits on one instruction
    ("Too many sync wait commands"). Split excess waits onto same-engine
    nops placed immediately before the over-subscribed instruction, and
    do the same for the Tile tail Drain."""
    if getattr(tile.TileContext, "_wait_split_applied", False):
        return

    orig_lower = tile.TileContext._lower_ordered_insts

    def _split_waits(self, ordered):
        for bb_name, insts in ordered.items():
            out = []
            for inst in insts:
                si = inst.sync_info
                if si is not None and len(si.on_wait) > _MAX_WAITS:
                    waits = list(si.on_wait)
                    rest, keep = waits[:-_MAX_WAITS], waits[-_MAX_WAITS:]
                    inst.sync_info = mybir.SyncInfo(
                        on_update=list(si.on_update), on_wait=keep
                    )
                    for i in range(0, len(rest), _MAX_WAITS):
                        out.append(
                            mybir.InstNoOp(
                                name=f"{inst.name}.wsplit{i}",
                                engine=inst.engine,
                                bass_nofuse=True,
                                sync_info=mybir.SyncInfo(
                                    on_update=[],
                                    on_wait=rest[i : i + _MAX_WAITS],
                                ),
                            )
                        )
                out.append(inst)
            ordered[bb_name] = out

    def _lower_ordered_insts(self, ordered):
        _split_waits(self, ordered)
        return orig_lower(self, ordered)

    def _drain_and_barrier(self, tick_clock, wait_clock):
        drain_inst = self.nc.sync.drain()
        wait_clock.add_sem_waits(
            drain_inst.ins, ScopedClock({None: tick_clock.global_clock})
        )
        waits = list(drain_inst.ins.sync_info.on_wait)
        if len(waits) > _MAX_WAITS:
            drain_inst.ins.sync_info.on_wait = waits[:_MAX_WAITS]
            rest = waits[_MAX_WAITS:]
            for i in range(0, len(rest), _MAX_WAITS):
                nop = self.nc.sync.nop(nofuse=True, hint="drain_wait_split")
                nop.ins.sync_info = mybir.SyncInfo(
                    on_update=[], on_wait=rest[i : i + _MAX_WAITS]
                )

        tail_mode = os.environ.get("BASS_TAIL_MODE", "slim")
        assert self.sems is not None
        popped = self.nc._tile_sem_poison_stack.pop()
        assert popped is self._sem_poison
        if tail_mode == "full":
            self.nc.all_engine_barrier()
            self.nc.clear_and_free_semaphores(list(self.sems.allocated().values()))
            self.nc.all_engine_barrier()
        elif tail_mode == "slim":
            # Engine streams end right after the clear; the next execute
            # of this NEFF can only be submitted after every stream (incl.
            # gpsimd's clears) has retired, so the trailing barrier is
            # redundant for a non-looping kernel.
            self.nc.all_engine_barrier()
            self.nc.clear_and_free_semaphores(list(self.sems.allocated().values()))
        elif tail_mode == "semonly":
            self.nc.all_engine_barrier(sem_only=True)
            self.nc.clear_and_free_semaphores(list(self.sems.allocated().values()))
        else:
            raise ValueError(f"unknown BASS_TAIL_MODE {tail_mode}")

    tile.TileContext._lower_ordered_insts = _lower_ordered_insts
    tile.TileContext._drain_and_barrier = _drain_and_barrier
    tile.TileContext._wait_split_applied = True


def _install_ntff_hook():
    """Register the axon NTFF profile hook (the image's antenv package lacks
    axon_hooks, so trace=True would silently degrade otherwise)."""
    if "antenv.axon_hooks" in sys.modules:
        return
    mod = types.ModuleType("antenv.axon_hooks")
    state = {"hook": None}
    mod.set_axon_ntff_profile_hook = lambda h: state.__setitem__("hook", h)
    mod.get_axon_ntff_profile_hook = lambda: state["hook"]
    sys.modules["antenv.axon_hooks"] = mod
    try:
        import antenv

        antenv.axon_hooks = mod
    except Exception:
        pass
    try:
        from trn_agent_boot.trn_boot import _ntff_profile_via_ctypes

        hook = _ntff_profile_via_ctypes("/opt/axon/libaxon_pjrt.so")
        if hook is not None:
            mod.set_axon_ntff_profile_hook(hook)
    except Exception:
        pass


_apply_tile_wait_split_patch()
_install_ntff_hook()


# ---------------------------------------------------------------------------
# Device kernel
# ---------------------------------------------------------------------------

ZW = BSH * RANK_Q          # z^T columns per core = 192
BF16 = mybir.dt.bfloat16


def _chunk_cols(c):
    return min(CH, N - CH * c)


def _phase_cols(p):
    return sum(_chunk_cols(PH_OFF[p] + i) for i in range(PH[p]))


def _build_nc():
    """Per core: z^T = sum_n sum_q x[(s,q), n] Lq_s[q, i] Ln_s[n, j].

    Stage 1 (per n-chunk c of 128): psum_T[n', (s,i)] accumulates
    x_tile_r^T @ BD_r over the 3 row-tiles r, where BD_r is the
    block-diagonal Lq_s for the samples covered by rows [128r, 128r+128).
    Samples straddling a row-tile boundary are summed by the PSUM
    accumulation. x tiles are the stationary operand in natural layout
    (no transposes; every PE op is a real matmul), converted to bf16 so
    LDWEIGHTS runs with FWL and overlaps in-flight matmuls.

    Stage 2: psum_z[j, (s,i)] accumulates lns_c^T @ T_c over the 32
    chunks. Stage-2 matmuls are emitted DELAY chunks behind stage 1 so
    the PSUM->SBUF copy of T_c is off the PE critical path.
    """
    nc = bass.Bass()
    x = nc.declare_dram_parameter("x", [ROWS, N], F32, isOutput=False)
    lns = nc.declare_dram_parameter("lns", [128, NCH * RANK_N], BF16, isOutput=False)
    bd = nc.declare_dram_parameter("bd", [128, RT * ZW], BF16, isOutput=False)
    zt = nc.declare_dram_parameter("zt", [RANK_N, ZW], F32, isOutput=True)
    rs = nc.declare_dram_parameter("rs", [128, RT * NPH], F32, isOutput=True)

    mult = mybir.AluOpType.mult
    DELAY = 3
    MAXPC = max(PH) * CH   # largest phase width in columns

    with tile.TileContext(nc) as tc, ExitStack() as ctx:
        const = ctx.enter_context(tc.tile_pool(name="const", bufs=1))
        bfp = [
            ctx.enter_context(tc.tile_pool(name=f"bf{r}", bufs=NPH))
            for r in range(RT)
        ]
        sqp = ctx.enter_context(tc.tile_pool(name="sq", bufs=2))
        ttp = ctx.enter_context(tc.tile_pool(name="tt", bufs=DELAY + 2))
        outp = ctx.enter_context(tc.tile_pool(name="outs", bufs=1))
        pt = ctx.enter_context(tc.tile_pool(name="pt", bufs=DELAY + 2, space="PSUM"))
        pz = ctx.enter_context(tc.tile_pool(name="pz", bufs=1, space="PSUM"))

        rs_sb = outp.tile([128, RT * NPH], F32)
        pzt = pz.tile([RANK_N, ZW], F32)
        pending = []  # (chunk, tt tile) awaiting the stage-2 matmul

        def stage2(c, tt):
            csz = _chunk_cols(c)
            nc.tensor.matmul(
                pzt[:],
                lns_sb[0:csz, RANK_N * c : RANK_N * (c + 1)],
                tt[0:csz, :],
                start=(c == 0),
                stop=(c == NCH - 1),
            )

        # Persistent bf16 image of x, one tile per 128-row tile; DMA phases
        # write column slices so triggers never wait on buffer recycling.
        # Phase-0 x loads go out first; constants follow (they are only
        # needed once the first matmuls run).
        bd_sb = const.tile([128, RT * ZW], BF16)
        lns_sb = const.tile([128, NCH * RANK_N], BF16)
        xbf = [[None] * NPH for _ in range(RT)]
        for r in range(RT):
            xb = bfp[r].tile([128, MAXPC], BF16, name=f"xb{r}_0", tag=f"xb{r}")
            nc.gpsimd.dma_start(
                xb[0:128, 0 : _phase_cols(0)],
                x[128 * r : 128 * (r + 1), 0 : _phase_cols(0)],
            )
            xbf[r][0] = xb
        nc.sync.dma_start(bd_sb[:], bd[:])
        nc.sync.dma_start(lns_sb[:], lns[:])

        # Warmup matmuls on constants: keep the PE busy through the DMA
        # ramp so the HAM clock gate opens (1.2 -> 2.4 GHz) before the
        # real matmuls start.
        n_warm = int(os.environ.get("BASS_WARM_MM", "24"))
        if n_warm:
            pj = pz.tile([128, 512], F32, tag="junk")
            for _ in range(n_warm):
                nc.tensor.matmul(
                    pj[:], bd_sb[:, 0:128], bd_sb[:, 0:512], start=True, stop=True
                )

        for p in range(NPH):
            pcols = _phase_cols(p)
            col0 = CH * PH_OFF[p]
            for r in range(RT):
                if p > 0:
                    # Casting DMA (SWDGE): f32 DRAM -> bf16 SBUF.
                    xb = bfp[r].tile([128, MAXPC], BF16, name=f"xb{r}_{p}", tag=f"xb{r}")
                    nc.gpsimd.dma_start(
                        xb[0:128, 0:pcols],
                        x[128 * r : 128 * (r + 1), col0 : col0 + pcols],
                    )
                    xbf[r][p] = xb
                xb = xbf[r][p]
                # x^2 (to scratch) + rowsum accumulator in one DVE op
                sq = sqp.tile([128, MAXPC], BF16)
                slot = r * NPH + p
                nc.vector.scalar_tensor_tensor(
                    sq[0:128, 0:pcols],
                    xb[0:128, 0:pcols],
                    1.0,
                    xb[0:128, 0:pcols],
                    mult,
                    mult,
                    accum_out=rs_sb[:, slot : slot + 1],
                )
            for cc in range(PH[p]):
                c = PH_OFF[p] + cc
                csz = _chunk_cols(c)
                ptc = pt.tile([CH, ZW], F32)
                for r in range(RT):
                    nc.tensor.matmul(
                        ptc[0:csz, :],
                        xbf[r][p][:, CH * cc : CH * cc + csz],
                        bd_sb[:, ZW * r : ZW * (r + 1)],
                        start=(r == 0),
                        stop=(r == RT - 1),
                    )
                tt = ttp.tile([CH, ZW], BF16)
                # PSUM->SBUF copies on ScalarE (otherwise mostly idle).
                nc.scalar.copy(tt[0:csz, :], ptc[0:csz, :])
                pending.append((c, tt))
                if len(pending) > DELAY:
                    stage2(*pending.pop(0))
        for c, tt in pending:
            stage2(c, tt)

        zto = outp.tile([RANK_N, ZW], F32, tag="zto")
        nc.scalar.copy(zto[:], pzt[:])
        nc.sync.dma_start(zt[:], zto[:])
        # Copy through DVE (program order after all accum writers) so the
        # DMA-out has a tracked producer for every element.
        rs_out = outp.tile([128, RT * NPH], F32, tag="rs_out")
        nc.vector.tensor_copy(rs_out[:], rs_sb[:])
        nc.sync.dma_start(rs[:], rs_out[:])
    return nc


_NC = None


def _get_nc():
    global _NC
    if _NC is None:
        _NC = _build_nc()
    return _NC


# ---------------------------------------------------------------------------
# Host wrapper
# ---------------------------------------------------------------------------

def kernel(eps_t, y_t, L_n, L_q, sigma):
    global LAST_EXEC_TIME_NS
    eps_t = np.ascontiguousarray(eps_t, dtype=np.float32)
    y_t = np.ascontiguousarray(y_t, dtype=np.float32)
    L_n = np.asarray(L_n, dtype=np.float32)
    L_q = np.asarray(L_q, dtype=np.float32)
    sigma = np.asarray(sigma, dtype=np.float32)
    assert eps_t.shape == (B, Q, N) and y_t.shape == (B, Q, N)

    import ml_dtypes

    lns = np.ascontiguousarray(L_n / np.float32(np.sqrt(RANK_N)))
    lqs32 = (L_q / np.float32(np.sqrt(RANK_Q))).astype(np.float32)
    lqs = lqs32.astype(np.float64)

    # lns row-packed into chunks of 128: lnp[p, 30c + j] = lns[128c + p, j]
    lnp = np.zeros((128, NCH * RANK_N), dtype=np.float32)
    for c in range(NCH):
        csz = _chunk_cols(c)
        lnp[:csz, RANK_N * c : RANK_N * (c + 1)] = lns[CH * c : CH * c + csz]
    lnp = lnp.astype(ml_dtypes.bfloat16)

    # Block-diagonal Lq_s per 128-row tile: bd[p, r*ZW + s*12 + i] =
    # lqs[q, i] where 128r + p = 24s + q (sample-local rows).
    bdm = np.zeros((128, RT * ZW), dtype=np.float32)
    for r in range(RT):
        for p in range(128):
            g = 128 * r + p
            s, q = divmod(g, Q)
            bdm[p, r * ZW + s * RANK_Q : r * ZW + (s + 1) * RANK_Q] = lqs32[q]
    bdm = bdm.astype(ml_dtypes.bfloat16)

    # The reference masks x where y_t is exactly 0.0f. y_t is randn-filled,
    # so this never fires in practice; handle the degenerate case on the
    # host so the device only has to stream x.
    if np.any(y_t == 0.0):
        eps_t = eps_t * (y_t != 0.0).astype(np.float32)

    xf = eps_t.reshape(B * Q, N)
    in_maps = [
        {
            "x": np.ascontiguousarray(xf[i * ROWS : (i + 1) * ROWS]),
            "lns": lnp,
            "bd": bdm,
        }
        for i in range(NCORES)
    ]

    nc = _get_nc()
    trace = bool(os.environ.get("BASS_KERNEL_TRACE"))
    res = run_bass_kernel_spmd(nc, in_maps, list(range(NCORES)), trace=trace)
    if trace:
        LAST_EXEC_TIME_NS = res.exec_time_ns

    # Gather z [B, R] (device zt is [30, (s, i)] per core) and row sums.
    z = np.concatenate(
        [
            res.results[i]["zt"]
            .astype(np.float64)
            .reshape(RANK_N, BSH, RANK_Q)
            .transpose(1, 2, 0)
            .reshape(BSH, RANK_Q * RANK_N)
            for i in range(NCORES)
        ]
    )
    rows = np.concatenate(
        [
            res.results[i]["rs"].reshape(128, RT, NPH).sum(axis=2).T.reshape(ROWS)
            for i in range(NCORES)
        ]
    )

    return _host_finish(z, rows, lqs, lns.astype(np.float64), sigma)


def _host_finish(z, rows, lqs, lns64, sigma):
    """Tiny O(R^3) finish in float64. z: [B, R]; rows: [B*Q] sums of
    masked x^2; lqs/lns64: scaled cov factors in float64."""
    D = Q * N
    R = RANK_Q * RANK_N

    s2 = rows.astype(np.float64).reshape(B, Q).sum(axis=1)

    # Capacitance grams: A = lqs^T lqs (rq x rq), Bm = lns^T lns (rn x rn).
    A = lqs.T @ lqs
    Bm = lns64.T @ lns64

    diag_bias = np.log(np.expm1(np.float64(SIGMA_INIT**2)))
    c = np.logaddexp(0.0, np.float64(sigma[0]) + diag_bias) + SIGMA_MIN**2

    cap = np.eye(R) + np.kron(A, Bm) / c
    L = np.linalg.cholesky(cap)
    logdet = 2.0 * np.sum(np.log(np.diagonal(L))) + D * np.log(c)

    try:
        from scipy.linalg import solve_triangular

        u = solve_triangular(L, z.T, lower=True)
    except Exception:
        u = np.linalg.solve(L, z.T)
    maha = s2 / c - (u * u).sum(axis=0) / (c * c)

    loss = np.mean(0.5 * (D * np.log(2.0 * np.pi) + logdet + maha))
    return np.float32(loss)


# revision 49
# speedup vs baseline: 1.1872x; 1.0428x over previous
"""Trainium2 Bass kernel for the low-rank MGD (Mahalanobis Gaussian) loss.

Strategy (data-parallel over batch across 8 NeuronCores):
  - Each core receives a [384, 4000] shard of x (384 = 16 samples x 24
    q-rows) and computes, fully on device, its samples' Mahalanobis
    ingredients: per-row sums of x^2 (fused DVE multiply-reduce) and
    z^T[j, (s,i)] = sum_{n,q} x[(s,q),n] Lq_s[q,i] Ln_s[n,j] via two
    PSUM-accumulated matmul stages (x as the bf16 stationary operand
    against a block-diagonal Lq_s, then Ln_s^T against the stage-1
    output). No transposes are needed anywhere.
  - The y_t != 0 mask is handled on the host: y_t is randn-filled, so it
    contains an exact f32 zero with probability ~0; kernel() verifies that
    and falls back to masking x on the host in the degenerate case. The
    device therefore only streams x (49MB instead of 98MB).
  - Host gathers the tiny per-core outputs (z [B, 360] and row sums) and
    finishes: the 360x360 capacitance cholesky / logdet / triangular
    solve, and the final scalar loss. This is ~30 MFLOP of O(R^3) linear
    algebra on 47KB of data - negligible next to what the device streams.
"""

import os
import sys
import types
from contextlib import ExitStack

import numpy as np

if "/opt/trn_rl_repo" not in sys.path:
    sys.path.insert(0, "/opt/trn_rl_repo")

import concourse.bass as bass
import concourse.tile as tile
import concourse.mybir as mybir
from concourse.bass_utils import run_bass_kernel_spmd
from concourse.vector_clock import ScopedClock

F32 = mybir.dt.float32

# Problem constants (hardcoded per the harness contract).
B, Q, N = 128, 24, 4000
RANK_N, RANK_Q = 30, 12
SIGMA_INIT = 1.0
SIGMA_MIN = 0.001
NCORES = 8
BSH = B // NCORES          # samples per core = 16
ROWS = BSH * Q             # (b, q) rows per core = 384
RT = ROWS // 128           # 128-row tiles per core = 3
NCH = 32                   # matmul n-chunks of 128 (last 32)
CH = 128
# chunks per phase: small first (fast pipeline fill), big in the middle
# (few triggers at steady state), small last (fast drain)
PH = [4, 8, 8, 8, 2, 1, 1]
NPH = len(PH)
PH_OFF = [sum(PH[:i]) for i in range(NPH)]      # first chunk of each phase

LAST_EXEC_TIME_NS = None


# ---------------------------------------------------------------------------
# Environment fixups
# ---------------------------------------------------------------------------

_MAX_WAITS = 1  # walrus codegen here rejects multiple sync-waits on one instruction


def _apply_tile_wait_split_patch():
    """walrus in this image rejects >2 sync-waits on one instruction
    ("Too many sync wait commands"). Split excess waits onto same-engine
    nops placed immediately before the over-subscribed instruction, and
    do the same for the Tile tail Drain."""
    if getattr(tile.TileContext, "_wait_split_applied", False):
        return

    orig_lower = tile.TileContext._lower_ordered_insts

    def _split_waits(self, ordered):
        for bb_name, insts in ordered.items():
            out = []
            for inst in insts:
                si = inst.sync_info
                if si is not None and len(si.on_wait) > _MAX_WAITS:
                    waits = list(si.on_wait)
                    rest, keep = waits[:-_MAX_WAITS], waits[-_MAX_WAITS:]
                    inst.sync_info = mybir.SyncInfo(
                        on_update=list(si.on_update), on_wait=keep
                    )
                    for i in range(0, len(rest), _MAX_WAITS):
                        out.append(
                            mybir.InstNoOp(
                                name=f"{inst.name}.wsplit{i}",
                                engine=inst.engine,
                                bass_nofuse=True,
                                sync_info=mybir.SyncInfo(
                                    on_update=[],
                                    on_wait=rest[i : i + _MAX_WAITS],
                                ),
                            )
                        )
                out.append(inst)
            ordered[bb_name] = out

    def _lower_ordered_insts(self, ordered):
        _split_waits(self, ordered)
        return orig_lower(self, ordered)

    def _drain_and_barrier(self, tick_clock, wait_clock):
        drain_inst = self.nc.sync.drain()
        wait_clock.add_sem_waits(
            drain_inst.ins, ScopedClock({None: tick_clock.global_clock})
        )
        waits = list(drain_inst.ins.sync_info.on_wait)
        if len(waits) > _MAX_WAITS:
            drain_inst.ins.sync_info.on_wait = waits[:_MAX_WAITS]
            rest = waits[_MAX_WAITS:]
            for i in range(0, len(rest), _MAX_WAITS):
                nop = self.nc.sync.nop(nofuse=True, hint="drain_wait_split")
                nop.ins.sync_info = mybir.SyncInfo(
                    on_update=[], on_wait=rest[i : i + _MAX_WAITS]
                )

        tail_mode = os.environ.get("BASS_TAIL_MODE", "slim")
        assert self.sems is not None
        popped = self.nc._tile_sem_poison_stack.pop()
        assert popped is self._sem_poison
        if tail_mode == "full":
            self.nc.all_engine_barrier()
            self.nc.clear_and_free_semaphores(list(self.sems.allocated().values()))
            self.nc.all_engine_barrier()
        elif tail_mode == "slim":
            # Engine streams end right after the clear; the next execute
            # of this NEFF can only be submitted after every stream (incl.
            # gpsimd's clears) has retired, so the trailing barrier is
            # redundant for a non-looping kernel.
            self.nc.all_engine_barrier()
            self.nc.clear_and_free_semaphores(list(self.sems.allocated().values()))
        elif tail_mode == "semonly":
            self.nc.all_engine_barrier(sem_only=True)
            self.nc.clear_and_free_semaphores(list(self.sems.allocated().values()))
        else:
            raise ValueError(f"unknown BASS_TAIL_MODE {tail_mode}")

    tile.TileContext._lower_ordered_insts = _lower_ordered_insts
    tile.TileContext._drain_and_barrier = _drain_and_barrier
    tile.TileContext._wait_split_applied = True


def _install_ntff_hook():
    """Register the axon NTFF profile hook (the image's antenv package lacks
    axon_hooks, so trace=True would silently degrade otherwise)."""
    if "antenv.axon_hooks" in sys.modules:
        return
    mod = types.ModuleType("antenv.axon_hooks")
    state = {"hook": None}
    mod.set_axon_ntff_profile_hook = lambda h: state.__setitem__("hook", h)
    mod.get_axon_ntff_profile_hook = lambda: state["hook"]
    sys.modules["antenv.axon_hooks"] = mod
    try:
        import antenv

        antenv.axon_hooks = mod
    except Exception:
        pass
    try:
        from trn_agent_boot.trn_boot import _ntff_profile_via_ctypes

        hook = _ntff_profile_via_ctypes("/opt/axon/libaxon_pjrt.so")
        if hook is not None:
            mod.set_axon_ntff_profile_hook(hook)
    except Exception:
        pass


_apply_tile_wait_split_patch()
_install_ntff_hook()


# ---------------------------------------------------------------------------
# Device kernel
# ---------------------------------------------------------------------------

ZW = BSH * RANK_Q          # z^T columns per core = 192
BF16 = mybir.dt.bfloat16


def _chunk_cols(c):
    return min(CH, N - CH * c)


def _phase_cols(p):
    return sum(_chunk_cols(PH_OFF[p] + i) for i in range(PH[p]))


def _build_nc():
    """Per core: z^T = sum_n sum_q x[(s,q), n] Lq_s[q, i] Ln_s[n, j].

    Stage 1 (per n-chunk c of 128): psum_T[n', (s,i)] accumulates
    x_tile_r^T @ BD_r over the 3 row-tiles r, where BD_r is the
    block-diagonal Lq_s for the samples covered by rows [128r, 128r+128).
    Samples straddling a row-tile boundary are summed by the PSUM
    accumulation. x tiles are the stationary operand in natural layout
    (no transposes; every PE op is a real matmul), converted to bf16 so
    LDWEIGHTS runs with FWL and overlaps in-flight matmuls.

    Stage 2: psum_z[j, (s,i)] accumulates lns_c^T @ T_c over the 32
    chunks. Stage-2 matmuls are emitted DELAY chunks behind stage 1 so
    the PSUM->SBUF copy of T_c is off the PE critical path.
    """
    nc = bass.Bass()
    x = nc.declare_dram_parameter("x", [ROWS, N], F32, isOutput=False)
    lns = nc.declare_dram_parameter("lns", [128, NCH * RANK_N], BF16, isOutput=False)
    bd = nc.declare_dram_parameter("bd", [128, RT * ZW], BF16, isOutput=False)
    zt = nc.declare_dram_parameter("zt", [RANK_N, ZW], F32, isOutput=True)
    rs = nc.declare_dram_parameter("rs", [128, RT * NPH], F32, isOutput=True)

    mult = mybir.AluOpType.mult
    DELAY = 4
    MAXPC = max(PH) * CH   # largest phase width in columns

    with tile.TileContext(nc) as tc, ExitStack() as ctx:
        const = ctx.enter_context(tc.tile_pool(name="const", bufs=1))
        bfp = [
            ctx.enter_context(tc.tile_pool(name=f"bf{r}", bufs=NPH))
            for r in range(RT)
        ]
        sqp = ctx.enter_context(tc.tile_pool(name="sq", bufs=2))
        ttp = ctx.enter_context(tc.tile_pool(name="tt", bufs=DELAY + 2))
        outp = ctx.enter_context(tc.tile_pool(name="outs", bufs=1))
        pt = ctx.enter_context(tc.tile_pool(name="pt", bufs=DELAY + 2, space="PSUM"))
        pz = ctx.enter_context(tc.tile_pool(name="pz", bufs=1, space="PSUM"))

        rs_sb = outp.tile([128, RT * NPH], F32)
        pzt = pz.tile([RANK_N, ZW], F32)
        pending = []  # (chunk, tt tile) awaiting the stage-2 matmul

        def stage2(c, tt):
            csz = _chunk_cols(c)
            nc.tensor.matmul(
                pzt[:],
                lns_sb[0:csz, RANK_N * c : RANK_N * (c + 1)],
                tt[0:csz, :],
                start=(c == 0),
                stop=(c == NCH - 1),
            )

        # Persistent bf16 image of x, one tile per 128-row tile; DMA phases
        # write column slices so triggers never wait on buffer recycling.
        # Phase-0 x loads go out first; constants follow (they are only
        # needed once the first matmuls run).
        bd_sb = const.tile([128, RT * ZW], BF16)
        lns_sb = const.tile([128, NCH * RANK_N], BF16)
        xbf = [[None] * NPH for _ in range(RT)]
        for r in range(RT):
            xb = bfp[r].tile([128, MAXPC], BF16, name=f"xb{r}_0", tag=f"xb{r}")
            nc.gpsimd.dma_start(
                xb[0:128, 0 : _phase_cols(0)],
                x[128 * r : 128 * (r + 1), 0 : _phase_cols(0)],
            )
            xbf[r][0] = xb
        nc.sync.dma_start(bd_sb[:], bd[:])
        nc.sync.dma_start(lns_sb[:], lns[:])

        # Warmup matmuls on constants: keep the PE busy through the DMA
        # ramp so the HAM clock gate opens (1.2 -> 2.4 GHz) before the
        # real matmuls start.
        n_warm = int(os.environ.get("BASS_WARM_MM", "24"))
        if n_warm:
            pj = pz.tile([128, 512], F32, tag="junk")
            for _ in range(n_warm):
                nc.tensor.matmul(
                    pj[:], bd_sb[:, 0:128], bd_sb[:, 0:512], start=True, stop=True
                )

        for p in range(NPH):
            pcols = _phase_cols(p)
            col0 = CH * PH_OFF[p]
            for r in range(RT):
                if p > 0:
                    # Casting DMA (SWDGE): f32 DRAM -> bf16 SBUF.
                    xb = bfp[r].tile([128, MAXPC], BF16, name=f"xb{r}_{p}", tag=f"xb{r}")
                    nc.gpsimd.dma_start(
                        xb[0:128, 0:pcols],
                        x[128 * r : 128 * (r + 1), col0 : col0 + pcols],
                    )
                    xbf[r][p] = xb
                xb = xbf[r][p]
                # x^2 (to scratch) + rowsum accumulator in one DVE op
                sq = sqp.tile([128, MAXPC], BF16)
                slot = r * NPH + p
                nc.vector.scalar_tensor_tensor(
                    sq[0:128, 0:pcols],
                    xb[0:128, 0:pcols],
                    1.0,
                    xb[0:128, 0:pcols],
                    mult,
                    mult,
                    accum_out=rs_sb[:, slot : slot + 1],
                )
            for cc in range(PH[p]):
                c = PH_OFF[p] + cc
                csz = _chunk_cols(c)
                ptc = pt.tile([CH, ZW], F32)
                for r in range(RT):
                    nc.tensor.matmul(
                        ptc[0:csz, :],
                        xbf[r][p][:, CH * cc : CH * cc + csz],
                        bd_sb[:, ZW * r : ZW * (r + 1)],
                        start=(r == 0),
                        stop=(r == RT - 1),
                    )
                tt = ttp.tile([CH, ZW], BF16)
                # PSUM->SBUF copies on ScalarE (otherwise mostly idle).
                nc.scalar.copy(tt[0:csz, :], ptc[0:csz, :])
                pending.append((c, tt))
                if len(pending) > DELAY:
                    stage2(*pending.pop(0))
        for c, tt in pending:
            stage2(c, tt)

        zto = outp.tile([RANK_N, ZW], F32, tag="zto")
        nc.scalar.copy(zto[:], pzt[:])
        nc.sync.dma_start(zt[:], zto[:])
        # Copy through DVE (program order after all accum writers) so the
        # DMA-out has a tracked producer for every element.
        rs_out = outp.tile([128, RT * NPH], F32, tag="rs_out")
        nc.vector.tensor_copy(rs_out[:], rs_sb[:])
        nc.sync.dma_start(rs[:], rs_out[:])
    return nc


_NC = None


def _get_nc():
    global _NC
    if _NC is None:
        _NC = _build_nc()
    return _NC


# ---------------------------------------------------------------------------
# Host wrapper
# ---------------------------------------------------------------------------

def kernel(eps_t, y_t, L_n, L_q, sigma):
    global LAST_EXEC_TIME_NS
    eps_t = np.ascontiguousarray(eps_t, dtype=np.float32)
    y_t = np.ascontiguousarray(y_t, dtype=np.float32)
    L_n = np.asarray(L_n, dtype=np.float32)
    L_q = np.asarray(L_q, dtype=np.float32)
    sigma = np.asarray(sigma, dtype=np.float32)
    assert eps_t.shape == (B, Q, N) and y_t.shape == (B, Q, N)

    import ml_dtypes

    lns = np.ascontiguousarray(L_n / np.float32(np.sqrt(RANK_N)))
    lqs32 = (L_q / np.float32(np.sqrt(RANK_Q))).astype(np.float32)
    lqs = lqs32.astype(np.float64)

    # lns row-packed into chunks of 128: lnp[p, 30c + j] = lns[128c + p, j]
    lnp = np.zeros((128, NCH * RANK_N), dtype=np.float32)
    for c in range(NCH):
        csz = _chunk_cols(c)
        lnp[:csz, RANK_N * c : RANK_N * (c + 1)] = lns[CH * c : CH * c + csz]
    lnp = lnp.astype(ml_dtypes.bfloat16)

    # Block-diagonal Lq_s per 128-row tile: bd[p, r*ZW + s*12 + i] =
    # lqs[q, i] where 128r + p = 24s + q (sample-local rows).
    bdm = np.zeros((128, RT * ZW), dtype=np.float32)
    for r in range(RT):
        for p in range(128):
            g = 128 * r + p
            s, q = divmod(g, Q)
            bdm[p, r * ZW + s * RANK_Q : r * ZW + (s + 1) * RANK_Q] = lqs32[q]
    bdm = bdm.astype(ml_dtypes.bfloat16)

    # The reference masks x where y_t is exactly 0.0f. y_t is randn-filled,
    # so this never fires in practice; handle the degenerate case on the
    # host so the device only has to stream x.
    if np.any(y_t == 0.0):
        eps_t = eps_t * (y_t != 0.0).astype(np.float32)

    xf = eps_t.reshape(B * Q, N)
    in_maps = [
        {
            "x": np.ascontiguousarray(xf[i * ROWS : (i + 1) * ROWS]),
            "lns": lnp,
            "bd": bdm,
        }
        for i in range(NCORES)
    ]

    nc = _get_nc()
    trace = bool(os.environ.get("BASS_KERNEL_TRACE"))
    res = run_bass_kernel_spmd(nc, in_maps, list(range(NCORES)), trace=trace)
    if trace:
        LAST_EXEC_TIME_NS = res.exec_time_ns

    # Gather z [B, R] (device zt is [30, (s, i)] per core) and row sums.
    z = np.concatenate(
        [
            res.results[i]["zt"]
            .astype(np.float64)
            .reshape(RANK_N, BSH, RANK_Q)
            .transpose(1, 2, 0)
            .reshape(BSH, RANK_Q * RANK_N)
            for i in range(NCORES)
        ]
    )
    rows = np.concatenate(
        [
            res.results[i]["rs"].reshape(128, RT, NPH).sum(axis=2).T.reshape(ROWS)
            for i in range(NCORES)
        ]
    )

    return _host_finish(z, rows, lqs, lns.astype(np.float64), sigma)


def _host_finish(z, rows, lqs, lns64, sigma):
    """Tiny O(R^3) finish in float64. z: [B, R]; rows: [B*Q] sums of
    masked x^2; lqs/lns64: scaled cov factors in float64."""
    D = Q * N
    R = RANK_Q * RANK_N

    s2 = rows.astype(np.float64).reshape(B, Q).sum(axis=1)

    # Capacitance grams: A = lqs^T lqs (rq x rq), Bm = lns^T lns (rn x rn).
    A = lqs.T @ lqs
    Bm = lns64.T @ lns64

    diag_bias = np.log(np.expm1(np.float64(SIGMA_INIT**2)))
    c = np.logaddexp(0.0, np.float64(sigma[0]) + diag_bias) + SIGMA_MIN**2

    cap = np.eye(R) + np.kron(A, Bm) / c
    L = np.linalg.cholesky(cap)
    logdet = 2.0 * np.sum(np.log(np.diagonal(L))) + D * np.log(c)

    try:
        from scipy.linalg import solve_triangular

        u = solve_triangular(L, z.T, lower=True)
    except Exception:
        u = np.linalg.solve(L, z.T)
    maha = s2 / c - (u * u).sum(axis=0) / (c * c)

    loss = np.mean(0.5 * (D * np.log(2.0 * np.pi) + logdet + maha))
    return np.float32(loss)
